# revision 8
# baseline (speedup 1.0000x reference)
"""Causal self-attention TRN2 Bass kernel (bf16 pipeline, v2).

Problem (hardcoded): B=2, S=2048, D=1024, H=16, DK=64, fp32 I/O.
  Q = einsum('bsd,hdk->bhsk', x, Wq); K, V likewise
  scores = Q K^T / sqrt(DK), causal mask, softmax
  out = (softmax @ V) concat heads @ Wo^T

Sharding: 8 cores = 2 batches x 4 head-groups (4 heads each).
Each core computes QKV projections for its 4 heads, attention, and the
partial Wo product for its 256 columns of the concat dim.  The host sums
the 4 partials per batch (tensor-parallel all-reduce at unshard time)
and transposes the (D, S) partial back to (S, D).

v2 changes vs baseline (all driven by the v1 CoreSim cost model):
  - whole data path in bf16 (PSUM accumulation stays f32): halves DMA
    bytes; matmul stays 1 cycle/row.
  - input DMA split fine-grained (weights first, x per (st, c-pair))
    across the SP/Pool/DVE queues so the first matmul starts ~4us in
    instead of ~11us.  Output partials are copied PSUM->SBUF as bf16 on
    Pool and DMA'd by SP; DMA never rides the ACT engine (exp is the
    attention-phase bottleneck).
  - crossing-tile exp and causal-mask affine_select each merged into a
    single 2-range-AP instruction (halves their instruction count).
  - software-pipelined attention emission: scores(c+1) is emitted before
    AV(c), and projection / Wo matmuls are woven between attention
    chunks as PE fill while ACT (exp) is the per-chunk rate limiter.
  - PSUM: stp ring 2x[128,1024] + ot ring 2 + acc ring 2x[128,512]
    = 8 banks exactly.
"""

import numpy as np
import ml_dtypes

import concourse.bacc as bacc
import concourse.mybir as mybir
import concourse.tile as tile
from concourse.bass_utils import run_bass_kernel_spmd

BF16 = ml_dtypes.bfloat16

B, S, D, H, DK = 2, 2048, 1024, 16, 64
NCORES = 8
GROUPS = 4  # head groups per batch
HL = 4  # heads per core
NPAIR = 2  # head pairs per core
DC = D // 128  # 8 contraction chunks
SC = S // 128  # 16 key chunks
QT = S // 512  # 4 query tiles
NEGC = -12.0  # softmax shift: weights = exp(score - 12) / sum

MM_DT = mybir.dt.bfloat16

_PROG = None


def _emit(nc, xT, wqk, wv, wo, outT):
    f32 = mybir.dt.float32
    AF = mybir.ActivationFunctionType
    Alu = mybir.AluOpType

    tc = nc._tc  # set by _build

    with (
        tc.tile_pool(name="big", bufs=2) as big,
        tc.tile_pool(name="wgt", bufs=1) as wgt,
        tc.tile_pool(name="nrm", bufs=2) as nrm,
        tc.tile_pool(name="stg", bufs=3) as stg,
        tc.tile_pool(name="ps_st", bufs=2, space="PSUM") as ps_st,
        tc.tile_pool(name="ps_ot", bufs=2, space="PSUM") as ps_ot,
        tc.tile_pool(name="ps_acc", bufs=2, space="PSUM") as ps_acc,
    ):
        # ---------- input DMA ----------
        # x arrives pre-transposed, grouped by query/key 512-tile then
        # d-chunk: x_one[p, st*DC*512 + c*512 + s'] = x[b].T[c*128+p, st*512+s']
        # Queue plan (DMA transfer time occupies the issuing engine in the
        # cost model, so keep it off ACT, and off DVE once compute starts):
        #   SP:   wqk[0,0], x st0 (4 c-pair chunks), x st1, later all outputs
        #   Pool: x st2, x st3, wv, wo
        #   DVE:  wqk[1,0], wqk[0,1], wqk[1,1]  (small, before DVE compute)
        x_one = big.tile([128, DC * S], MM_DT, tag="xbig", bufs=1, name="x_one")

        def x_chunk(engine, st, cp):
            g = DC * 512
            lo = st * g + cp * 1024
            engine.dma_start(out=x_one[:, lo : lo + 1024], in_=xT[:, lo : lo + 1024])

        wqk_sb = {}
        for qk in range(2):
            for p in range(NPAIR):
                t = wgt.tile(
                    [128, DC * 128], MM_DT, tag=f"wqk{qk}{p}", name=f"wqk{qk}{p}"
                )
                wqk_sb[qk, p] = t

        nc.sync.dma_start(out=wqk_sb[0, 0][:], in_=wqk[0, 0])
        for st in range(QT):
            eng = nc.sync if st < 2 else nc.gpsimd
            for cp in range(4):
                x_chunk(eng, st, cp)
        # ACT is idle until the first exp (~20us in): safe for small loads
        for qk, p in ((1, 0), (0, 1), (1, 1)):
            nc.scalar.dma_start(out=wqk_sb[qk, p][:], in_=wqk[qk, p])

        wv_sb = wgt.tile([128, DC * HL * DK], MM_DT, tag="wv", name="wv_sb")
        nc.gpsimd.dma_start(out=wv_sb[:], in_=wv[:])

        wo_sb = []
        for p in range(NPAIR):
            t = wgt.tile([128, D], MM_DT, tag=f"wo{p}", name=f"wo{p}")
            nc.gpsimd.dma_start(out=t[:], in_=wo[p])
            wo_sb.append(t)

        def xcol(c, s0, n):
            """x^T[c*128:(c+1)*128, s0:s0+n] — n must stay in one 512 tile."""
            st, s_ = divmod(s0, 512)
            assert s_ + n <= 512
            base = st * DC * 512 + c * 512 + s_
            return x_one[:, base : base + n]

        # V buffer: [128, SC * HL * 65]; per key-chunk, per head: 64 V
        # columns + a ones column (for the softmax denominator row).
        v_sb = wgt.tile([128, SC * HL * 65], MM_DT, tag="v", name="v_sb")
        v_view = v_sb[:].rearrange("p (c h k) -> p c h k", c=SC, h=HL)
        nc.vector.memset(v_view[:, :, :, 64:65], 1.0)

        # per-partition constant for the exp bias
        neg_c = wgt.tile([128, 1], f32, tag="negc", name="neg_c")
        nc.vector.memset(neg_c[:], NEGC)

        qt_sb = []
        kt_sb = []
        concat_sb = []
        for p in range(NPAIR):
            qt_sb.append(wgt.tile([128, S], MM_DT, tag=f"qt{p}", name=f"qt{p}"))
            kt_sb.append(wgt.tile([128, S], MM_DT, tag=f"kt{p}", name=f"kt{p}"))
            concat_sb.append(
                big.tile([128, S], MM_DT, tag="big", name=f"concat{p}")
            )

        # ---------- emission units ----------
        def qk_group(p, qk, st):
            # one 512-query tile of the Q^T (or K^T) pair projection
            dest = qt_sb[p] if qk == 0 else kt_sb[p]
            w = wqk_sb[qk, p]
            ps = ps_acc.tile([128, 512], f32, tag="acc", name="proj_ps")
            for c in range(DC):
                nc.tensor.matmul(
                    ps[:],
                    w[:, c * 128 : (c + 1) * 128],
                    xcol(c, st * 512, 512),
                    start=(c == 0),
                    stop=(c == DC - 1),
                )
            nc.vector.tensor_copy(dest[:, st * 512 : (st + 1) * 512], ps[:])

        def v_chunk(sc):
            # V natural layout for all 4 heads of one key chunk
            ps = ps_acc.tile([128, HL * DK], f32, tag="acc", name="vproj_ps")
            for c in range(DC):
                nc.tensor.matmul(
                    ps[:],
                    xcol(c, sc * 128, 128),
                    wv_sb[:, c * 256 : (c + 1) * 256],
                    start=(c == 0),
                    stop=(c == DC - 1),
                )
            nc.vector.tensor_copy(
                v_view[:, sc, :, 0:64], ps[:].rearrange("p (h k) -> p h k", h=HL)
            )

        # attention state per (pair): ot accumulators
        class AttnState:
            __slots__ = ("ot_a", "ot_b")

        astate = {}

        def attn_start(p, qt):
            st_ = AttnState()
            st_.ot_a = ps_ot.tile([65, 512], f32, tag="ot", name="ot_a")
            st_.ot_b = ps_ot.tile([65, 512], f32, tag="ot", name="ot_b")
            astate[p] = st_

        def attn_scores(p, qt, sc):
            # scores^T + exp + mask for one (pair, query-tile, key-chunk)
            d = max(0, sc * 128 - qt * 512)
            q0 = qt * 512 + d
            stp = ps_st.tile([128, 1024], f32, tag="st", name="st_ps")
            nc.tensor.matmul(
                stp[:, d:512],
                kt_sb[p][0:64, sc * 128 : (sc + 1) * 128],
                qt_sb[p][0:64, q0 : (qt + 1) * 512],
                start=True,
                stop=True,
                tile_position=(0, 0),
            )
            nc.tensor.matmul(
                stp[:, 512 + d : 1024],
                kt_sb[p][64:128, sc * 128 : (sc + 1) * 128],
                qt_sb[p][64:128, q0 : (qt + 1) * 512],
                start=True,
                stop=True,
                tile_position=(64, 0),
            )
            pt = big.tile([128, 1024], MM_DT, tag="pt", bufs=3, name="pt")
            if d == 0:
                nc.scalar.activation(pt[:], stp[:], AF.Exp, bias=neg_c[:])
            else:
                # one activation over both head halves via a 2-range AP
                pt3 = pt[:].rearrange("p (j q) -> p j q", j=2)
                st3 = stp[:].rearrange("p (j q) -> p j q", j=2)
                nc.scalar.activation(
                    pt3[:, :, d:512], st3[:, :, d:512], AF.Exp, bias=neg_c[:]
                )
            if sc >= 4 * qt:
                # zero where key s = sc*128+part exceeds query
                # q = qt*512+d+q_loc  (predicate: q_loc >= part), both
                # head halves in one 2-range instruction
                pt3 = pt[:].rearrange("p (j q) -> p j q", j=2)
                n = 512 - d
                nc.gpsimd.affine_select(
                    pt3[:, :, d:512],
                    pt3[:, :, d:512],
                    pattern=[[0, 2], [1, n]],
                    base=0,
                    channel_multiplier=-1,
                    compare_op=Alu.is_ge,
                    fill=0.0,
                )
            return (sc, d, pt)

        def attn_av(p, chunk, nvalid):
            # AV matmuls for a chunk produced by attn_scores
            st_ = astate[p]
            sc, d, pt = chunk
            nc.tensor.matmul(
                st_.ot_a[:, d:512],
                v_view[:, sc, 2 * p, :],
                pt[:, d:512],
                start=(sc == 0),
                stop=(sc == nvalid - 1),
            )
            nc.tensor.matmul(
                st_.ot_b[:, d:512],
                v_view[:, sc, 2 * p + 1, :],
                pt[:, 512 + d : 1024],
                start=(sc == 0),
                stop=(sc == nvalid - 1),
            )

        def attn_norm(p, qt):
            # normalize into concat^T pair tile (odd head at rows 64:128)
            st_ = astate[p]
            for half, ot in ((0, st_.ot_a), (1, st_.ot_b)):
                drow = nrm.tile([1, 512], f32, tag="drow", name="drow")
                nc.vector.tensor_copy(drow[:], ot[64:65, :])
                erow = nrm.tile([1, 512], f32, tag="erow", name="erow")
                nc.vector.reciprocal_approx_fast(erow[:], drow[:])
                ebc = nrm.tile([64, 512], f32, tag="ebc", name="ebc")
                nc.gpsimd.partition_broadcast(ebc[:], erow[:])
                nc.vector.tensor_mul(
                    concat_sb[p][64 * half : 64 * half + 64,
                                 qt * 512 : (qt + 1) * 512],
                    ot[0:64, :],
                    ebc[:],
                )

        dma_rr = [0]
        out_engines = [nc.sync, nc.sync, nc.gpsimd]

        def wo_block(qt, co):
            # partial^T[co, qt] = sum over pairs of wo_pair^T @ concat_pair
            ps = ps_acc.tile([128, 512], f32, tag="acc", name="wo_ps")
            for p in range(NPAIR):
                nc.tensor.matmul(
                    ps[:],
                    wo_sb[p][:, co * 128 : (co + 1) * 128],
                    concat_sb[p][:, qt * 512 : (qt + 1) * 512],
                    start=(p == 0),
                    stop=(p == NPAIR - 1),
                )
            # GPSIMD cannot access PSUM (walrus birverifier) — DVE copies
            ob = stg.tile([128, 512], MM_DT, tag="ob", name="ob")
            nc.vector.tensor_copy(ob[:], ps[:])
            eng = out_engines[dma_rr[0] % len(out_engines)]
            dma_rr[0] += 1
            eng.dma_start(
                out=outT[co][:, qt * 512 : (qt + 1) * 512], in_=ob[:]
            )

        # ---------- schedule ----------
        def attn_qt(p, qt, fills):
            """Software-pipelined chunk emission for one (pair, qtile):
            the PE stream is s0, (s1 a0), (s2 a1), ..., a_last, so AV(c)
            only dispatches after scores(c+1) — exp(c) has a full chunk
            of PE work (plus fills) to complete without stalling PE.

            fills: list of (deadline, thunk).  A fill is emitted no later
            than the end of step `deadline` (before AV(deadline) enters
            the PE stream) — required for v_chunk(sc) fills consumed by
            this qtile's own AV(sc).  Otherwise fills are paced evenly.
            """
            nvalid = 4 * (qt + 1)
            attn_start(p, qt)
            fi = 0
            nsteps = nvalid + 1

            def do_fills(i):
                nonlocal fi
                while fi < len(fills) and (
                    fills[fi][0] <= i
                    or fi + 1 <= (i + 1) * len(fills) / nsteps
                ):
                    fills[fi][1]()
                    fi += 1

            prev = attn_scores(p, qt, 0)
            do_fills(0)
            for sc in range(1, nvalid):
                cur = attn_scores(p, qt, sc)
                attn_av(p, prev, nvalid)
                prev = cur
                do_fills(sc)
            attn_av(p, prev, nvalid)
            do_fills(nsteps)
            attn_norm(p, qt)

        LATE = 10**6

        # S1: pair-0 Q/K projections
        for st in range(QT):
            qk_group(0, 0, st)
            qk_group(0, 1, st)
        # S2: V for key chunks 0..3
        for sc in range(4):
            v_chunk(sc)
        # S3: attn0 qt0 woven with pair-1 Q/K projections
        qk1_fills = [
            (LATE, lambda qk=qk, st=st: qk_group(1, qk, st))
            for st in range(QT)
            for qk in range(2)
        ]
        attn_qt(0, 0, qk1_fills[:4])
        # S4-S6: attn0 qt1..3 woven with remaining qk1 + V chunks 4..15.
        # v_chunk(sc) must be emitted before this qtile's AV(sc).
        rest = qk1_fills[4:]

        def v_fills(lo):
            return [(sc - 1, lambda sc=sc: v_chunk(sc)) for sc in range(lo, lo + 4)]

        attn_qt(0, 1, v_fills(4) + rest[:2])
        attn_qt(0, 2, v_fills(8) + rest[2:])
        attn_qt(0, 3, v_fills(12))
        # S7: attn1 qt0
        attn_qt(1, 0, [])
        # S8-S10: attn1 qt1..3 woven with wo(0..2)
        attn_qt(1, 1, [(LATE, lambda co=co: wo_block(0, co)) for co in range(DC)])
        attn_qt(1, 2, [(LATE, lambda co=co: wo_block(1, co)) for co in range(DC)])
        attn_qt(1, 3, [(LATE, lambda co=co: wo_block(2, co)) for co in range(DC)])
        # S11: final wo
        for co in range(DC):
            wo_block(3, co)


def _build():
    nc = bacc.Bacc("TRN2", target_bir_lowering=False, debug=False)
    mdt = MM_DT
    xT = nc.dram_tensor("xT", [128, DC * S], mdt, kind="ExternalInput").ap()
    wqk = nc.dram_tensor(
        "wqk", [2, NPAIR, 128, DC * 128], mdt, kind="ExternalInput"
    ).ap()
    wv = nc.dram_tensor("wv", [128, DC * HL * DK], mdt, kind="ExternalInput").ap()
    wo = nc.dram_tensor("wo", [NPAIR, 128, D], mdt, kind="ExternalInput").ap()
    outT = nc.dram_tensor("outT", [DC, 128, S], mdt, kind="ExternalOutput").ap()
    with tile.TileContext(nc) as tc:
        nc._tc = tc
        _emit(nc, xT, wqk, wv, wo, outT)
    nc.compile()
    return nc


def get_program():
    global _PROG
    if _PROG is None:
        _PROG = _build()
    return _PROG


def make_in_maps(x, Wq, Wk, Wv, Wo):
    x = np.asarray(x, np.float32)
    Wq = np.asarray(Wq, np.float32)
    Wk = np.asarray(Wk, np.float32)
    Wv = np.asarray(Wv, np.float32)
    Wo = np.asarray(Wo, np.float32)
    in_maps = []
    for core in range(NCORES):
        b, g = divmod(core, GROUPS)
        hs = slice(HL * g, HL * g + HL)
        # [partition, (512-tile group, d-chunk, 512)]
        xT = np.ascontiguousarray(
            x[b].T.reshape(DC, 128, QT, 512).transpose(1, 2, 0, 3)
            .reshape(128, DC * S)
        ).astype(BF16)
        # SBUF layout [partition=d%128, (chunk, pair-col)]
        wqk = np.empty((2, NPAIR, 128, DC * 128), np.float32)
        for i, W in enumerate((Wq, Wk)):
            Wl = W[hs]
            if i == 0:
                Wl = Wl * np.float32(1.0 / np.sqrt(DK))  # exact (2^-3)
            for p in range(NPAIR):
                wqk3 = wqk[i, p].reshape(128, DC, 128)
                wqk3[:, :, 0:DK] = Wl[2 * p].reshape(DC, 128, DK).transpose(1, 0, 2)
                wqk3[:, :, DK:128] = (
                    Wl[2 * p + 1].reshape(DC, 128, DK).transpose(1, 0, 2)
                )
        wv = np.ascontiguousarray(
            Wv[hs].transpose(1, 0, 2).reshape(D, HL * DK)  # (D, 256)
            .reshape(DC, 128, HL * DK).transpose(1, 0, 2)  # (128, DC, 256)
            .reshape(128, DC * HL * DK)
        ).astype(BF16)
        wo = np.ascontiguousarray(
            Wo[:, 256 * g : 256 * (g + 1)].T
        ).reshape(NPAIR, 128, D).astype(BF16)
        in_maps.append(
            {"xT": xT, "wqk": wqk.astype(BF16), "wv": wv, "wo": wo}
        )
    return in_maps


def combine_outputs(per_core_outT):
    """per_core_outT: list of 8 arrays shaped (DC,128,S) -> full (B,S,D)."""
    out = np.empty((B, S, D), np.float32)
    for b in range(B):
        acc = np.zeros((D, S), np.float32)
        for g in range(GROUPS):
            acc += per_core_outT[GROUPS * b + g].reshape(D, S).astype(np.float32)
        out[b] = acc.T
    return out


def kernel(x, Wq, Wk, Wv, Wo):
    nc = get_program()
    in_maps = make_in_maps(x, Wq, Wk, Wv, Wo)
    res = run_bass_kernel_spmd(nc, in_maps, list(range(NCORES)))
    return combine_outputs([r["outT"] for r in res.results])


if __name__ == "__main__":
    rng = np.random.default_rng(0)
    x = rng.standard_normal((B, S, D), dtype=np.float32)
    sc = np.float32(1.0 / np.sqrt(D))
    Wq = rng.standard_normal((H, D, DK), dtype=np.float32) * sc
    Wk = rng.standard_normal((H, D, DK), dtype=np.float32) * sc
    Wv = rng.standard_normal((H, D, DK), dtype=np.float32) * sc
    Wo = rng.standard_normal((D, D), dtype=np.float32) * sc
    out = kernel(x, Wq, Wk, Wv, Wo)
    print("out", out.shape, out.dtype, float(np.abs(out).mean()))
